# revision 25
# baseline (speedup 1.0000x reference)
"""Trainium2 8-core Bass kernel for nn_Atom_Inter_Layer (GNN attention message passing).

Strategy: edges sharded by destination-node range (core c owns nodes
[1250c, 1250(c+1)) and all edges whose dst lands there), so segment
softmax/sum stay core-local — no collectives. Host does index bucketing,
selector-matrix staging and weight folding; all NN compute runs on device.

v2 redesign (vs the exp-decomposed baseline):
  - single ACT table (silu_and_others): mid-LN+SiLU fused into ONE scalar-engine
    Silu op with per-partition scale/bias; score exp synthesized exactly as
    exp(x) = (1+t)/(1-t), t = tanh(x/2) (scores are tiny, |x|<0.3).
  - per-edge 1/std via quake-style rsqrt (bit-trick seed + 1 Newton) batched
    over a whole block on the DVE (7 small ops per 33 subtiles).
  - A'[src] gather-add done on the PE (identity-matmul accumulate onto the
    same PSUM region, start=False) instead of DVE adds.
  - q[dst] folded into the B-table S-matmul (one [v|k|q] 1024-wide PSUM tile).
  - dst selector matrices S (edge-major + node-major) staged from host.
  - layout order [v(512) | k(256) | q(256)] so each matmul out stays in-bank.

Per-core pipeline: prep A'/Bq tables; then per 125-node block: phase A
(L1 matmuls + bn_stats + PSUM->SBUF copy per 128-edge subtile), batched
rsqrt chain, phase B (Silu, PE transposes, L2 matmuls, scores, scatter),
then the alpha-normalize + output-MLP epilogue.
"""
import sys

if "/opt/trn_rl_repo" not in sys.path:
    sys.path.insert(0, "/opt/trn_rl_repo")

from contextlib import ExitStack

import numpy as np
import ml_dtypes

import concourse.bass as bass
import concourse.bacc as bacc
import concourse.tile as tile
import bass_rust as _bass_rust
from concourse.hw_specs import get_activation_tables as _gat


def _patched_iatl(self):
    import concourse.mybir as _mb
    has_activation = any(
        isinstance(i, _mb.InstActivation)
        for b in self.main_func.blocks
        for i in b.instructions
    )
    if not has_activation:
        return
    tables = list(_gat(self.m.arch).items())
    if _FILTER_TABLES:
        keep = "silu_and_others"
        import concourse.mybir as _mb2
        drop = {_mb2.ActivationFunctionType.Silu, _mb2.ActivationFunctionType.Tanh,
                _mb2.ActivationFunctionType.Copy, _mb2.ActivationFunctionType.Identity}
        tables = [(n, (set(fns) if n == keep else {f for f in fns if f not in drop}))
                  for n, fns in tables]
    _bass_rust.insert_act_table_loads(self, tables)


_FILTER_TABLES = True
bacc.Bacc.insert_act_table_loads = _patched_iatl
from concourse import mybir
from concourse.bass_utils import run_bass_kernel_spmd

BF16 = mybir.dt.bfloat16
F32 = mybir.dt.float32
I32 = mybir.dt.int32
I16 = mybir.dt.int16

N, E, D, EDIM, H, C = 10000, 320000, 256, 64, 8, 32
FEAT = 2 * D + EDIM  # 576
NCORE = 8
NB = 10            # node blocks per core
BLK = 125          # nodes per block
NPC = NB * BLK     # nodes per core = 1250
NT = (N + 127) // 128          # 79 tiles of the full node table
NPAD = NT * 128                # 10112
EPS = 1e-5
AW = 1024          # augmented L1-psum row width: [v 512 | k 256 | q 256]
TW = 768           # A-table row width: [v 512 | k 256]
GCH = 9            # gather chunk (subtiles per dma_gather)
MAGIC = 0x5F3759DF
ISQ = 1.0 / np.sqrt(C)
USE_DMAT = True    # transpose via DMA xbar instead of PE+copy

bf16 = ml_dtypes.bfloat16


def _b(a):
    return np.ascontiguousarray(np.asarray(a, np.float32)).astype(bf16)


def host_prep(inputs):
    """Build per-core in_maps + static shapes from the full inputs."""
    x = np.asarray(inputs["x"], np.float32)
    ei = np.asarray(inputs["edge_index"]).astype(np.int64)
    ea = np.asarray(inputs["edge_attr"], np.float32)
    src, dst = ei[0], ei[1]
    g = np.asarray(inputs["ln_norm_g"], np.float32)
    kw1 = np.asarray(inputs["k_w1"], np.float32)
    vw1 = np.asarray(inputs["v_w1"], np.float32)

    # --- fast-path validity (biases zero / gains one folded trivially) ---
    for nm in ("ln_norm_b", "k_b1", "v_b1", "q_b1", "q_b2", "k_b2", "v_b2",
               "o_b1", "o_b2", "q_be", "k_be", "v_be"):
        assert np.abs(np.asarray(inputs[nm])).max() == 0.0, f"{nm} nonzero; fast path invalid"
    for nm in ("q_g", "k_g", "v_g"):
        assert np.abs(np.asarray(inputs[nm]) - 1.0).max() == 0.0, f"{nm} != 1"

    # --- weight folds (f32 host math); output order [v(512) | k(256)] ---
    v1cat = np.concatenate([g @ vw1, g @ kw1])                     # [768]
    rk1 = v1cat[None, :] / FEAT   # rank-1 LN-mean fold
    gk = g[:, None]
    wEA = np.concatenate([gk[:64] * vw1[0:64], gk[:64] * kw1[0:64]], 1) - rk1      # [64,768]
    wA = np.concatenate([gk[64:320] * vw1[64:320], gk[64:320] * kw1[64:320]], 1) - rk1
    wB = np.concatenate([gk[320:576] * vw1[320:576], gk[320:576] * kw1[320:576]], 1) - rk1
    wEAf = np.zeros((EDIM, AW), np.float32)
    wEAf[:, 0:768] = wEA
    wAf = np.zeros((D, AW), np.float32)
    wAf[:, 0:768] = wA

    # --- edge bucketing by destination block ---
    bucket = (dst // BLK).astype(np.int64)           # 0..79
    order = np.argsort(bucket, kind="stable")
    counts = np.bincount(bucket, minlength=NCORE * NB)
    nsub = int(np.ceil(counts.max() / 128))
    tblk = nsub * 128
    starts = np.zeros(NCORE * NB, np.int64)
    starts[1:] = np.cumsum(counts)[:-1]
    pos_in_blk = np.arange(E, dtype=np.int64) - starts[bucket[order]]

    idx_pad = np.zeros((NCORE * NB, tblk), np.int32)          # src gather index (pad -> 0)
    dst_pad = np.full((NCORE * NB, tblk), -1.0, np.float32)   # block-local dst (pad -> -1)
    ea_pad = np.zeros((NCORE * NB, tblk, EDIM), np.float32)
    bo = bucket[order]
    idx_pad[bo, pos_in_blk] = src[order].astype(np.int32)
    dst_pad[bo, pos_in_blk] = (dst[order] - bo * BLK).astype(np.float32)
    ea_pad[bo, pos_in_blk, :] = ea[order]

    # eaT: [core][64, NB*tblk], column order (block, t, p)
    eaT = ea_pad.reshape(NCORE, NB * tblk, EDIM).transpose(0, 2, 1)
    # dma_gather int16 indices: idx i at [i%16, i//16], replicated to 128 partitions
    idx16 = idx_pad.astype(np.int16).reshape(NCORE, NB, tblk // 16, 16).transpose(0, 1, 3, 2)
    idx16 = np.broadcast_to(idx16[:, :, None, :, :], (NCORE, NB, 8, 16, tblk // 16))
    idx16 = np.ascontiguousarray(idx16).reshape(NCORE, NB, 128, tblk // 16)

    # selector matrices: S[c,b,t,p,n] = (dst_local(edge p of subtile t) == n)
    dstb = dst_pad.reshape(NCORE, NB, nsub, 128)
    S = (dstb[..., None] == np.arange(128, dtype=np.float32)).astype(bf16)
    Seb = np.ascontiguousarray(S.transpose(0, 1, 3, 2, 4))   # [c, b, p(edge), t, n]
    Sne = np.ascontiguousarray(S.transpose(0, 1, 4, 2, 3))   # [c, b, n, t, p(edge)]

    # node-table layouts
    xpad = np.zeros((NPAD, D), np.float32)
    xpad[:N] = x
    xTfull = _b(xpad.T.reshape(D, NPAD))                    # [256, 10112]
    xTblk = x.reshape(NCORE, NB, BLK, D)
    xTb = np.zeros((NCORE, D, NB, 128), np.float32)
    xTb[:, :, :, :BLK] = xTblk.transpose(0, 3, 1, 2)

    ident = np.eye(128, dtype=np.float32)

    # wkv2: chunks 0..3 act on v-hidden -> out cols 256:512; 4..5 on k-hidden -> 0:256
    wkv2 = np.concatenate([
        np.asarray(inputs["v_w2"], np.float32).reshape(4, 128, 256),
        np.asarray(inputs["k_w2"], np.float32).reshape(2, 128, 256),
    ], 0)                                                    # [6,128,256]

    shapes = dict(nsub=nsub, tblk=tblk)
    common = {
        "wEAx": _b(wEAf),
        "wA": _b(wAf.reshape(2, 128, AW)),
        "wB": _b(wB.reshape(2, 128, 768)),
        "wq1": _b(np.asarray(inputs["q_w1"], np.float32).reshape(2, 128, 512)),
        "wq2": _b(np.asarray(inputs["q_w2"], np.float32).reshape(4, 128, 256)),
        "wkv2": _b(wkv2),
        "wo1": _b(np.asarray(inputs["o_w1"], np.float32).reshape(2, 128, 512)),
        "wo2": _b(np.asarray(inputs["o_w2"], np.float32).reshape(4, 128, 256)),
        "ident": _b(ident),
        "xTfull": xTfull,
    }
    in_maps = []
    for c in range(NCORE):
        m = dict(common)
        m["eaT"] = _b(eaT[c])
        m["idx"] = np.ascontiguousarray(idx16[c])
        m["Seb"] = np.ascontiguousarray(Seb[c])
        m["Sne"] = np.ascontiguousarray(Sne[c])
        m["xTb"] = _b(xTb[c].reshape(D, NB * 128))
        in_maps.append(m)
    return in_maps, shapes


def build(nsub, tblk, debug=False, finalize=True):
    """Build the single-core Bass graph (same on all 8 cores)."""
    nc = bacc.Bacc()
    p_eaT = nc.declare_dram_parameter("eaT", [EDIM, NB * tblk], BF16, isOutput=False)
    p_idx = nc.declare_dram_parameter("idx", [NB, 128, tblk // 16], I16, isOutput=False)
    p_Seb = nc.declare_dram_parameter("Seb", [NB, 128, nsub, 128], BF16, isOutput=False)
    p_Sne = nc.declare_dram_parameter("Sne", [NB, 128, nsub, 128], BF16, isOutput=False)
    p_xTb = nc.declare_dram_parameter("xTb", [D, NB * 128], BF16, isOutput=False)
    p_xTf = nc.declare_dram_parameter("xTfull", [D, NPAD], BF16, isOutput=False)
    p_wEAx = nc.declare_dram_parameter("wEAx", [EDIM, AW], BF16, isOutput=False)
    p_wA = nc.declare_dram_parameter("wA", [2, 128, AW], BF16, isOutput=False)
    p_wB = nc.declare_dram_parameter("wB", [2, 128, 768], BF16, isOutput=False)
    p_wq1 = nc.declare_dram_parameter("wq1", [2, 128, 512], BF16, isOutput=False)
    p_wq2 = nc.declare_dram_parameter("wq2", [4, 128, 256], BF16, isOutput=False)
    p_wkv2 = nc.declare_dram_parameter("wkv2", [6, 128, 256], BF16, isOutput=False)
    p_wo1 = nc.declare_dram_parameter("wo1", [2, 128, 512], BF16, isOutput=False)
    p_wo2 = nc.declare_dram_parameter("wo2", [4, 128, 256], BF16, isOutput=False)
    p_ident = nc.declare_dram_parameter("ident", [128, 128], BF16, isOutput=False)
    p_out = nc.declare_dram_parameter("out", [NPC, D], F32, isOutput=True)
    p_dbg = nc.declare_dram_parameter("dbg", [8, 128, AW], F32, isOutput=True) if debug else None
    A_dram = nc.dram_tensor("A_tab", [NPAD, AW], BF16)

    with tile.TileContext(nc) as tc, ExitStack() as ctx:
        const = ctx.enter_context(tc.tile_pool(name="const", bufs=1))
        persist = ctx.enter_context(tc.tile_pool(name="persist", bufs=1))
        # psum pools: ppT 2x2 banks + ppKV 2x1 + ppY 1 + ppA 1 = 8 banks
        ppT = ctx.enter_context(tc.tile_pool(name="ppT", bufs=2, space="PSUM"))
        ppKV = ctx.enter_context(tc.tile_pool(name="ppKV", bufs=2, space="PSUM"))
        ppY = ctx.enter_context(tc.tile_pool(name="ppY", bufs=1, space="PSUM"))
        ppA = ctx.enter_context(tc.tile_pool(name="ppA", bufs=1, space="PSUM"))
        # sbuf pools
        sp_g = ctx.enter_context(tc.tile_pool(name="sp_g", bufs=2))      # gather chunks
        sp_blk = ctx.enter_context(tc.tile_pool(name="sp_blk", bufs=2))  # per-block loads
        sp_h = ctx.enter_context(tc.tile_pool(name="sp_h", bufs=1))      # h1q per block
        sp_s = ctx.enter_context(tc.tile_pool(name="sp_s", bufs=3))      # s / sT tiles
        sp_w = ctx.enter_context(tc.tile_pool(name="sp_w", bufs=3))      # prod / m_ext
        sp_t = ctx.enter_context(tc.tile_pool(name="sp_t", bufs=4))      # small f32
        sp_c = ctx.enter_context(tc.tile_pool(name="sp_c", bufs=2))      # chain tiles
        sp_o = ctx.enter_context(tc.tile_pool(name="sp_o", bufs=2))      # outputs / A rows

        def cload(param, shape, dtype=BF16, rearr=None, **rkw):
            t = const.tile(shape, dtype, tag=param.name)
            src = param[:]
            if rearr:
                src = src.rearrange(rearr, **rkw)
            nc.sync.dma_start(out=t[:], in_=src)
            return t

        wEAx = cload(p_wEAx, [EDIM, AW])
        wA = cload(p_wA, [128, 2, AW], rearr="j p c -> p j c")
        wB = cload(p_wB, [128, 2, 768], rearr="j p c -> p j c")
        wq1 = cload(p_wq1, [128, 2, 512], rearr="j p c -> p j c")
        wq2 = cload(p_wq2, [128, 4, 256], rearr="j p c -> p j c")
        wkv2 = cload(p_wkv2, [128, 6, 256], rearr="j p c -> p j c")
        wo1 = cload(p_wo1, [128, 2, 512], rearr="j p c -> p j c")
        wo2 = cload(p_wo2, [128, 4, 256], rearr="j p c -> p j c")
        ident = cload(p_ident, [128, 128])
        xTb = cload(p_xTb, [128, 2, NB * 128], rearr="(j p) n -> p j n", p=128)

        Bq_sb = persist.tile([128, NB, AW], BF16)

        def rsqrt_chain(mean_ap, var_ap, nlane, rs_t, b2_t):
            """rs = rsqrt(var+eps), b2 = -mean*rs via quake seed + 1 Newton.
            mean_ap/var_ap: [128, nlane] f32 APs; rs_t/b2_t: packed [128, nlane]."""
            ve = sp_c.tile([128, nlane], F32, tag="ve")
            nc.vector.tensor_scalar(out=ve[:], in0=var_ap, scalar1=EPS, scalar2=None,
                                    op0=mybir.AluOpType.add)
            t1 = sp_c.tile([128, nlane], I32, tag="t1")
            nc.vector.tensor_scalar(out=t1[:], in0=ve[:].bitcast(I32), scalar1=1,
                                    scalar2=None, op0=mybir.AluOpType.arith_shift_right)
            y0 = sp_c.tile([128, nlane], I32, tag="y0")
            nc.vector.tensor_scalar(out=y0[:], in0=t1[:], scalar1=-1, scalar2=MAGIC,
                                    op0=mybir.AluOpType.mult, op1=mybir.AluOpType.add)
            y0f = y0[:].bitcast(F32)
            p = sp_c.tile([128, nlane], F32, tag="p")
            nc.vector.tensor_tensor(out=p[:], in0=y0f, in1=y0f, op=mybir.AluOpType.mult)
            qq = sp_c.tile([128, nlane], F32, tag="qq")
            nc.vector.scalar_tensor_tensor(out=qq[:], in0=ve[:], scalar=-0.5, in1=p[:],
                                           op0=mybir.AluOpType.mult, op1=mybir.AluOpType.mult)
            nc.vector.scalar_tensor_tensor(out=rs_t[:], in0=qq[:], scalar=1.5, in1=y0f,
                                           op0=mybir.AluOpType.add, op1=mybir.AluOpType.mult)
            nc.vector.scalar_tensor_tensor(out=b2_t[:], in0=mean_ap, scalar=-1.0, in1=rs_t[:],
                                           op0=mybir.AluOpType.mult, op1=mybir.AluOpType.mult)

        # ================= PREP =================
        prep_stack = ExitStack()
        prepc = prep_stack.enter_context(tc.tile_pool(name="prepc", bufs=3))
        for i in range(NT):
            xTf = prepc.tile([128, 2, 128], BF16, tag="xTf")
            nc.sync.dma_start(out=xTf[:],
                              in_=p_xTf[:, i * 128:(i + 1) * 128]
                              .rearrange("(j p) n -> p j n", p=128))
            T = ppT.tile([128, AW], F32, tag="T")
            for j in range(2):
                nc.tensor.matmul(T[:, 0:512], xTf[:, j, :], wA[:, j, 0:512],
                                 start=(j == 0), stop=(j == 1))
            for j in range(2):
                nc.tensor.matmul(T[:, 512:1024], xTf[:, j, :], wA[:, j, 512:1024],
                                 start=(j == 0), stop=(j == 1))
            at = sp_o.tile([128, AW], BF16, tag="atab")
            nc.scalar.copy(out=at[:, 0:512], in_=T[:, 0:512])
            nc.vector.tensor_copy(out=at[:, 512:1024], in_=T[:, 512:1024])
            nc.gpsimd.dma_start(out=A_dram[i * 128:(i + 1) * 128, :], in_=at[:])

        for b in range(NB):
            # B' part
            T2 = ppT.tile([128, AW], F32, tag="T")
            for j in range(2):
                lhsT = xTb[:, j, b * 128:(b + 1) * 128]
                nc.tensor.matmul(T2[:, 0:512], lhsT, wB[:, j, 0:512],
                                 start=(j == 0), stop=(j == 1))
            for j in range(2):
                lhsT = xTb[:, j, b * 128:(b + 1) * 128]
                nc.tensor.matmul(T2[:, 512:768], lhsT, wB[:, j, 512:768],
                                 start=(j == 0), stop=(j == 1))
            nc.scalar.copy(out=Bq_sb[:, b, 0:512], in_=T2[:, 0:512])
            nc.vector.tensor_copy(out=Bq_sb[:, b, 512:768], in_=T2[:, 512:768])
            # q part: Linear -> LN -> SiLU -> Linear
            pq1 = ppKV.tile([128, 512], F32, tag="kv")
            for j in range(2):
                nc.tensor.matmul(pq1[:], xTb[:, j, b * 128:(b + 1) * 128], wq1[:, j, :],
                                 start=(j == 0), stop=(j == 1))
            stq = sp_t.tile([128, 6], F32, tag="stq")
            nc.vector.bn_stats(out=stq[:], in_=pq1[:])
            mvq = sp_t.tile([128, 2], F32, tag="mvq")
            nc.vector.bn_aggr(out=mvq[:], in_=stq[:])
            rsq = sp_c.tile([128, 1], F32, tag="rs")
            b2q = sp_c.tile([128, 1], F32, tag="b2")
            rsqrt_chain(mvq[:, 0:1], mvq[:, 1:2], 1, rsq, b2q)
            sq = sp_s.tile([128, 768], BF16, tag="s")
            nc.scalar.activation(out=sq[:, 0:512], in_=pq1[:],
                                 func=mybir.ActivationFunctionType.Silu,
                                 bias=b2q[:], scale=rsq[:])
            yTq = ppY.tile([128, 768], BF16, tag="yT")
            for i in range(4):
                nc.tensor.transpose(yTq[:, i * 128:(i + 1) * 128],
                                    sq[:, i * 128:(i + 1) * 128], ident[:])
            sqT = sp_s.tile([128, 768], BF16, tag="sT")
            nc.vector.tensor_copy(out=sqT[:, 0:512], in_=yTq[:, 0:512])
            pq2 = ppKV.tile([128, 512], F32, tag="kv")
            for i in range(4):
                nc.tensor.matmul(pq2[:, 0:256], sqT[:, i * 128:(i + 1) * 128], wq2[:, i, :],
                                 start=(i == 0), stop=(i == 3))
            nc.scalar.copy(out=Bq_sb[:, b, 768:1024], in_=pq2[:, 0:256])

        prep_stack.close()
        tc.strict_bb_all_engine_barrier()

        # ================= MAIN =================
        chunks = [(s, min(s + GCH, nsub)) for s in range(0, nsub, GCH)]

        for b in range(NB):
            idx_t = sp_blk.tile([128, tblk // 16], I16, tag="idx")
            nc.sync.dma_start(out=idx_t[:], in_=p_idx[b])
            acc = ppA.tile([128, 264], F32, tag="acc")
            cstate = {}

            def phaseA(ci):
                h0, h1c = chunks[ci]
                cnt = h1c - h0
                ag = sp_g.tile([128, GCH, AW], BF16, tag="ag")
                nc.gpsimd.dma_gather(
                    out_ap=ag[:, 0:cnt, :],
                    in_ap=A_dram[:],
                    idxs_ap=idx_t[:, h0 * 8:h1c * 8],
                    num_idxs=cnt * 128,
                    num_idxs_reg=cnt * 128,
                    elem_size=AW,
                    single_packet=False,
                )
                eaT_t = sp_g.tile([EDIM, GCH * 128], BF16, tag="ea")
                nc.sync.dma_start(out=eaT_t[:, 0:cnt * 128],
                                  in_=p_eaT[:, b * tblk + h0 * 128:b * tblk + h1c * 128])
                Sne_t = sp_g.tile([128, GCH, 128], BF16, tag="Sne")
                nc.sync.dma_start(out=Sne_t[:, 0:cnt, :], in_=p_Sne[b, :, h0:h1c, :])
                mv_c = sp_c.tile([128, GCH, 4], F32, tag="mv")
                h1q = sp_h.tile([128, GCH, AW], BF16, tag="h1q")
                cstate[ci] = (mv_c, h1q)
                for t in range(h0, h1c):
                    j = t - h0
                    T = ppT.tile([128, AW], F32, tag="T")
                    ea_l = eaT_t[:, j * 128:(j + 1) * 128]
                    S_ne = Sne_t[:, j, :]
                    nc.tensor.matmul(T[:, 0:512], ea_l, wEAx[:, 0:512],
                                     start=True, stop=False)
                    nc.tensor.matmul(T[:, 512:1024], ea_l, wEAx[:, 512:1024],
                                     start=True, stop=False)
                    nc.tensor.matmul(T[:, 0:512], S_ne, Bq_sb[:, b, 0:512],
                                     start=False, stop=False)
                    nc.tensor.matmul(T[:, 512:1024], S_ne, Bq_sb[:, b, 512:1024],
                                     start=False, stop=False)
                    nc.tensor.matmul(T[:, 0:512], ident[:], ag[:, j, 0:512],
                                     start=False, stop=True)
                    nc.tensor.matmul(T[:, 512:1024], ident[:], ag[:, j, 512:1024],
                                     start=False, stop=True)
                    st6 = sp_t.tile([128, 2, 6], F32, tag="st6")
                    nc.vector.bn_stats(out=st6[:, 0, :], in_=T[:, 0:512])
                    nc.vector.bn_aggr(out=mv_c[:, j, 0:2], in_=st6[:, 0, :])
                    nc.vector.bn_stats(out=st6[:, 1, :], in_=T[:, 512:768])
                    nc.vector.bn_aggr(out=mv_c[:, j, 2:4], in_=st6[:, 1, :])
                    nc.scalar.copy(out=h1q[:, j, 0:512], in_=T[:, 0:512])
                    nc.scalar.copy(out=h1q[:, j, 512:1024], in_=T[:, 512:1024])

            def phaseB(ci):
                h0, h1c = chunks[ci]
                cnt = h1c - h0
                mv_c, h1q = cstate.pop(ci)
                mvap = mv_c[:]
                mean_ap = bass.AP(tensor=mvap.tensor, offset=mvap.offset,
                                  ap=[mvap.ap[0], [4, cnt], [2, 2]])
                var_ap = bass.AP(tensor=mvap.tensor, offset=mvap.offset + 1,
                                 ap=[mvap.ap[0], [4, cnt], [2, 2]])
                rs_all = sp_c.tile([128, GCH, 2], F32, tag="rsa")
                b2_all = sp_c.tile([128, GCH, 2], F32, tag="b2a")
                rsqrt_chain(mean_ap, var_ap, cnt * 2,
                            rs_all[:, 0:cnt, :], b2_all[:, 0:cnt, :])
                Seb_t = sp_blk.tile([128, GCH, 128], BF16, tag="Seb")
                nc.sync.dma_start(out=Seb_t[:, 0:cnt, :], in_=p_Seb[b, :, h0:h1c, :])
                for t in range(h0, h1c):
                    jj = t - h0
                    s_sb = sp_s.tile([128, 768], BF16, tag="s")
                    nc.scalar.activation(out=s_sb[:, 0:512], in_=h1q[:, jj, 0:512],
                                         func=mybir.ActivationFunctionType.Silu,
                                         bias=b2_all[:, jj, 0:1], scale=rs_all[:, jj, 0:1])
                    nc.scalar.activation(out=s_sb[:, 512:768], in_=h1q[:, jj, 512:768],
                                         func=mybir.ActivationFunctionType.Silu,
                                         bias=b2_all[:, jj, 1:2], scale=rs_all[:, jj, 1:2])
                    yT = ppY.tile([128, 768], BF16, tag="yT")
                    for i in range(6):
                        nc.tensor.transpose(yT[:, i * 128:(i + 1) * 128],
                                            s_sb[:, i * 128:(i + 1) * 128], ident[:])
                    sT = sp_s.tile([128, 768], BF16, tag="sT")
                    nc.vector.tensor_copy(out=sT[:], in_=yT[:])
                    kv = ppKV.tile([128, 512], F32, tag="kv")
                    for i in range(4):
                        nc.tensor.matmul(kv[:, 256:512], sT[:, i * 128:(i + 1) * 128],
                                         wkv2[:, i, :], start=(i == 0), stop=(i == 3))
                    for i in range(2):
                        nc.tensor.matmul(kv[:, 0:256], sT[:, (4 + i) * 128:(5 + i) * 128],
                                         wkv2[:, 4 + i, :], start=(i == 0), stop=(i == 1))
                    prod = sp_w.tile([128, 8, 32], BF16, tag="prod")
                    nc.vector.tensor_tensor(
                        out=prod[:],
                        in0=kv[:, 0:256].rearrange("p (h c) -> p h c", h=8),
                        in1=h1q[:, jj, 768:1024].rearrange("p (h c) -> p h c", h=8),
                        op=mybir.AluOpType.mult)
                    sc = sp_t.tile([128, 8], F32, tag="sc")
                    nc.vector.tensor_reduce(out=sc[:], in_=prod[:],
                                            axis=mybir.AxisListType.X, op=mybir.AluOpType.add)
                    th = sp_t.tile([128, 8], F32, tag="th")
                    nc.scalar.activation(out=th[:], in_=sc[:],
                                         func=mybir.ActivationFunctionType.Tanh,
                                         bias=0.0, scale=float(ISQ * 0.5))
                    am = sp_t.tile([128, 8], F32, tag="am")
                    nc.vector.tensor_scalar(out=am[:], in0=th[:], scalar1=-1.0, scalar2=1.0,
                                            op0=mybir.AluOpType.mult, op1=mybir.AluOpType.add)
                    rm = sp_t.tile([128, 8], F32, tag="rm")
                    nc.vector.reciprocal_approx_fast(out=rm[:], in_=am[:])
                    m_ext = sp_w.tile([128, 264], BF16, tag="mext")
                    nc.vector.scalar_tensor_tensor(out=m_ext[:, 256:264], in0=th[:], scalar=1.0,
                                                   in1=rm[:], op0=mybir.AluOpType.add,
                                                   op1=mybir.AluOpType.mult)
                    u = m_ext[:, 256:264]
                    ubc = bass.AP(tensor=u.tensor, offset=u.offset,
                                  ap=[u.ap[0], u.ap[1], [0, 32]])
                    nc.vector.tensor_tensor(
                        out=m_ext[:, 0:256].rearrange("p (h c) -> p h c", h=8),
                        in0=kv[:, 256:512].rearrange("p (h c) -> p h c", h=8),
                        in1=ubc, op=mybir.AluOpType.mult)
                    nc.tensor.matmul(acc[:], Seb_t[:, jj, :], m_ext[:],
                                     start=(t == 0), stop=(t == nsub - 1))

            # software pipeline: phase A runs one chunk ahead of phase B
            phaseA(0)
            for ci in range(len(chunks)):
                if ci + 1 < len(chunks):
                    phaseA(ci + 1)
                phaseB(ci)

            # ---- block epilogue: alpha-normalize + output MLP ----
            accs = sp_w.tile([128, 264], F32, tag="accs")
            nc.scalar.copy(out=accs[:], in_=acc[:])
            dmx = sp_t.tile([128, 8], F32, tag="dmx")
            nc.vector.tensor_scalar_max(out=dmx[:], in0=accs[:, 256:264], scalar1=1e-30)
            rec = sp_t.tile([128, 8], F32, tag="rec")
            nc.vector.reciprocal_approx_fast(out=rec[:], in_=dmx[:])
            agg = sp_w.tile([128, 256], BF16, tag="agg")
            rap = rec[:]
            rbc = bass.AP(tensor=rap.tensor, offset=rap.offset,
                          ap=[rap.ap[0], rap.ap[1], [0, 32]])
            nc.vector.tensor_tensor(out=agg[:].rearrange("p (h c) -> p h c", h=8),
                                    in0=accs[:, 0:256].rearrange("p (h c) -> p h c", h=8),
                                    in1=rbc, op=mybir.AluOpType.mult)
            yT2 = ppY.tile([128, 768], BF16, tag="yT")
            for i in range(2):
                nc.tensor.transpose(yT2[:, i * 128:(i + 1) * 128],
                                    agg[:, i * 128:(i + 1) * 128], ident[:])
            aT = sp_s.tile([128, 768], BF16, tag="sT")
            nc.vector.tensor_copy(out=aT[:, 0:256], in_=yT2[:, 0:256])
            po1 = ppKV.tile([128, 512], F32, tag="kv")
            for i in range(2):
                nc.tensor.matmul(po1[:], aT[:, i * 128:(i + 1) * 128], wo1[:, i, :],
                                 start=(i == 0), stop=(i == 1))
            so = sp_s.tile([128, 768], BF16, tag="s")
            nc.scalar.activation(out=so[:, 0:512], in_=po1[:],
                                 func=mybir.ActivationFunctionType.Silu,
                                 bias=0.0, scale=1.0)
            yT3 = ppY.tile([128, 768], BF16, tag="yT")
            for i in range(4):
                nc.tensor.transpose(yT3[:, i * 128:(i + 1) * 128],
                                    so[:, i * 128:(i + 1) * 128], ident[:])
            soT = sp_s.tile([128, 768], BF16, tag="sT")
            nc.vector.tensor_copy(out=soT[:, 0:512], in_=yT3[:, 0:512])
            po2 = ppKV.tile([128, 512], F32, tag="kv")
            for i in range(4):
                nc.tensor.matmul(po2[:, 0:256], soT[:, i * 128:(i + 1) * 128], wo2[:, i, :],
                                 start=(i == 0), stop=(i == 3))
            outt = sp_o.tile([128, 256], F32, tag="outt")
            nc.scalar.copy(out=outt[:], in_=po2[:, 0:256])
            nc.sync.dma_start(out=p_out[b * BLK:(b + 1) * BLK, :], in_=outt[:BLK, :])

    if finalize:
        nc.finalize()
    return nc


_CACHE = {}


def _get_nc(nsub, tblk):
    key = (nsub, tblk)
    if key not in _CACHE:
        _CACHE[key] = build(nsub, tblk)
    return _CACHE[key]


def kernel_run(inputs, trace=False, **kw):
    in_maps, shapes = host_prep(inputs)
    nc = _get_nc(shapes["nsub"], shapes["tblk"])
    res = run_bass_kernel_spmd(nc, in_maps, core_ids=list(range(NCORE)), trace=trace, **kw)
    out = np.concatenate([np.asarray(res.results[c]["out"], np.float32) for c in range(NCORE)], 0)
    return out, res


def kernel(**inputs) -> np.ndarray:
    out, _ = kernel_run(inputs)
    return out
